# revision 23
# baseline (speedup 1.0000x reference)
"""GAT model Bass/Tile kernel for TRN2 (self-contained, supertile-batched).

Per core: 512 graphs as 256 pairs (128 nodes / 112 edges), 16 supertiles
of 16 pairs. Phase-batched emission keeps PE streaming: MM1 x16 ->
fused value+src-score gathers (g2e, drained to SBUF) -> dst-score tiny
matmuls with rank-1 edge-attr bias -> den/rd -> scatter streams. The
previous supertile's pooling matmuls fill den/rd dependency stalls.
One-hot masks and self-loop bias tables are host-precomputed.
"""
import numpy as np
from contextlib import ExitStack

import concourse.bass as bass
import concourse.tile as tile
from concourse import bacc, mybir
from concourse.bass_utils import run_bass_kernel_spmd

F32 = mybir.dt.float32
I32 = mybir.dt.int32

B, A, OBS = 4096, 8, 56
P = 64
H, HID, HC = 8, 32, 256
IN, OUT = 16, 2
NCORES = 8
GPC = B // NCORES          # graphs per core (512)
EPP = 2 * OBS              # edges per pair (112)
S = 16                     # pairs per supertile
ALU = mybir.AluOpType
ACTF = mybir.ActivationFunctionType

# sp PSUM column regions (f32)
AD_, DEN_, RD_ = 0, 128, 256

# engine assignment tables (tunable): A=scalar/ACT, V=vector/DVE, P=gpsimd
# NOTE: GPSIMD cannot access PSUM on TRN2 (BIR verifier) — PSUM reads
# (drains/relu) must stay on A/V; P only gets SBUF-to-SBUF work.
HASO_ENG = ["V"] * 8 + ["A"] * 8     # per-pair hAso drain
MRAW_ENG = ["A"] * 8 + ["V"] * 8     # per-pair msgraw drain (PSUM read)
MSG2_ENG = ["V"] * 8                 # per-duet edge-message multiply (SBUF)
MSG2_L1 = ["P", "V"] * 4             # L1 variant (GpSimd idle in L1)
MSGS_ENG = ["P"] * 8                 # per-duet self-message multiply (SBUF)
RELU_ENG = ["A", "V"] * 4            # per-duet relu drain (PSUM read)
RELU_L1 = ["A", "A", "A", "V"] * 2   # L1 variant (DVE busy with gvv)


def build(npairs: int, vdt=mybir.dt.bfloat16, num_devices: int = NCORES,
          pool_psum: bool = False):
    assert npairs % S == 0
    NST = npairs // S
    nc = bacc.Bacc("TRN2", target_bir_lowering=False, debug=False,
                   num_devices=num_devices)

    def din(name, shape, dt):
        return nc.dram_tensor(name, shape, dt, kind="ExternalInput").ap()

    xt = din("xt", [IN, npairs * 128], vdt)
    sblk_d = din("sblk", [128, npairs * EPP], vdt)
    dblk_d = din("dblk", [128, npairs * EPP], vdt)
    dtt_d = din("dtt", [EPP, npairs * 128], vdt)
    earow_d = din("earow", [1, npairs * EPP], vdt)
    saet_d = din("saet", [128, npairs * 24], F32)  # (pair, layer, head)
    wev_d = din("wev", [1, 24], vdt)               # (layer, head)
    waug1 = din("waug1", [IN, 272], vdt)
    waug2 = din("waug2", [128, 544], vdt)
    waug3 = din("waug3", [128, 544], vdt)
    fc1a = din("fc1a", [128, HC], vdt)
    fc1g = din("fc1g", [128, HC], vdt)
    fc1b = din("fc1b", [128, 1], F32)
    fc2w = din("fc2w", [128, OUT], vdt)
    fc2b = din("fc2b", [OUT, 1], F32)
    ident = din("ident", [128, 128], vdt)

    out_d = nc.dram_tensor("out", [OUT, npairs * 16], F32,
                           kind="ExternalOutput").ap()

    with tile.TileContext(nc) as tc, ExitStack() as ctx:
        cpool = ctx.enter_context(tc.tile_pool(name="const", bufs=1))
        wk = ctx.enter_context(tc.tile_pool(name="work", bufs=2))
        eb = ctx.enter_context(tc.tile_pool(name="edges", bufs=2))
        ps = ctx.enter_context(tc.tile_pool(name="psum", bufs=2, space="PSUM"))

        def cload(ap, tag):
            t = cpool.tile(list(ap.shape), ap.dtype, tag=tag)
            nc.sync.dma_start(t[:], ap[:, :])
            return t

        c_w1, c_w2, c_w3 = cload(waug1, "w1"), cload(waug2, "w2"), cload(waug3, "w3")
        c_fc1a, c_fc1g = cload(fc1a, "fc1a"), cload(fc1g, "fc1g")
        c_fc1b, c_fc2w, c_fc2b = cload(fc1b, "fc1b"), cload(fc2w, "fc2w"), cload(fc2b, "fc2b")
        c_id = cload(ident, "ident")
        c_wev = cload(wev_d, "wev")

        wchunks = {1: [c_w1[:, :]],
                   2: [c_w2[:, 0:272], c_w2[:, 272:544]],
                   3: [c_w3[:, 0:272], c_w3[:, 272:544]]}

        def copy_engine(code, dst, src, fb_idx=0):
            """PSUM->SBUF copy/cast on the given engine."""
            code2 = code if (pool_psum or code != "P") else ("A" if fb_idx % 2 == 0 else "V")
            if code2 == "A":
                nc.scalar.copy(dst, src)
            elif code2 == "V":
                nc.vector.tensor_copy(dst, src)
            else:
                nc.gpsimd.tensor_copy(dst, src)

        def relu_engine(code, dst, src, fb_idx=0):
            code2 = code if (pool_psum or code != "P") else ("A" if fb_idx % 2 == 0 else "V")
            if code2 == "A":
                nc.scalar.activation(dst, src, ACTF.Relu)
            elif code2 == "V":
                nc.vector.tensor_scalar(dst, src, 0.0, None, ALU.max)
            else:
                nc.gpsimd.tensor_scalar(dst, src, 0.0, None, ALU.max)

        def tt_engine(code, out, in0, in1, op):
            if code == "V":
                nc.vector.tensor_tensor(out, in0, in1, op)
            else:
                nc.gpsimd.tensor_tensor(out, in0, in1, op)

        # pooling state from the previous supertile, flushed during the
        # next supertile's L1 phase (or standalone at the end).
        pend = {}

        def pool_head(p):
            """Allocate pooling tiles for the pending supertile (no EW)."""
            gev_o = wk.tile([128, 64], F32, tag="gev_o")
            p["gev_o"] = gev_o
            p["zmlp"] = ps.tile([128, 288], F32, tag="sp", name="zmlp")

        def pool_gvv(p, d):
            """One graph-mean reduce (DVE), interleaved into L1 value loop."""
            gvv = p["gev_o"][:, :].rearrange("p (c pr g) -> p pr c g",
                                             c=2, g=2)
            nc.vector.tensor_reduce(
                gvv[:, 2 * d:2 * d + 2, :, :],
                p["hT"][d][:, :].rearrange(
                    "p (a b g n) -> p a b g n", a=2, b=2, g=2),
                mybir.AxisListType.X, ALU.add)

        def pool_gev_cast(p):
            gev_v = wk.tile([128, 64], vdt, tag="gev_v")
            nc.scalar.copy(gev_v[:], p["gev_o"][:])
            p["gev_v"] = gev_v

        def pool_agent(p):
            zmlp, hT_l3 = p["zmlp"], p["hT"]
            for d in range(S // 2):
                for c in range(2):
                    agent = hT_l3[d][:, :].rearrange(
                        "p (a b g n) -> p a b g n", a=2, b=2, g=2)[:, :, c, :, 0:8]
                    nc.tensor.matmul(zmlp[:, d * 32:(d + 1) * 32],
                                     c_fc1a[:, bass.ts(c, 128)], agent,
                                     start=(c == 0), stop=(c == 1))

        def pool_fc1g(p):
            zmlp = p["zmlp"]
            for c in range(2):
                nc.tensor.matmul(zmlp[:, 256:288],
                                 c_fc1g[:, bass.ts(c, 128)],
                                 p["gev_v"][:, c * 32:(c + 1) * 32],
                                 start=(c == 0), stop=(c == 1))
            zgb = wk.tile([128, 32], F32, tag="zgb")
            nc.vector.scalar_tensor_tensor(
                zgb[:], zmlp[:, 256:288], 1.0,
                c_fc1b[:, 0:1].broadcast_to([128, 32]), ALU.mult, ALU.add)
            zt = wk.tile([128, 256], F32, tag="zt")
            nc.vector.scalar_tensor_tensor(
                zt[:].rearrange("p (a b) -> p a b", a=32),
                zmlp[:, 0:256].rearrange("p (a b) -> p a b", a=32), 1.0,
                zgb[:][:, :, None].broadcast_to([128, 32, 8]),
                ALU.mult, ALU.add)
            zbat = wk.tile([128, 256], vdt, tag="zbat")
            nc.scalar.activation(zbat[:], zt[:], ACTF.Relu)
            p["zbat"] = zbat

        def pool_fc2(p):
            fc2o = ps.tile([OUT, 256], F32, tag="h2", bufs=4)
            nc.tensor.matmul(fc2o[:], c_fc2w[:, :], p["zbat"][:],
                             start=True, stop=True)
            outs = wk.tile([OUT, 256], F32, tag="outs")
            nc.vector.tensor_scalar(outs[:], fc2o[:], c_fc2b[:, 0:1], None,
                                    ALU.add)
            nc.sync.dma_start(out_d[:, p["st"] * 256:(p["st"] + 1) * 256],
                              outs[:])
            p.clear()

        for st in range(NST):
            p0 = st * S

            # ---- supertile input DMAs ----
            sblk = eb.tile([128, S * EPP], vdt, tag="sblk")
            nc.sync.dma_start(sblk[:], sblk_d[:, p0 * EPP:(p0 + S) * EPP])
            dblk = eb.tile([128, S * EPP], vdt, tag="dblk")
            nc.sync.dma_start(dblk[:], dblk_d[:, p0 * EPP:(p0 + S) * EPP])
            dtt = eb.tile([EPP, S * 128], vdt, tag="dtt")
            nc.sync.dma_start(dtt[:], dtt_d[:, p0 * 128:(p0 + S) * 128])
            xs = eb.tile([IN, S * 128], vdt, tag="xs")
            nc.sync.dma_start(xs[:], xt[:, p0 * 128:(p0 + S) * 128])
            earow = eb.tile([1, S * EPP], vdt, tag="earow")
            nc.sync.dma_start(earow[:], earow_d[:, p0 * EPP:(p0 + S) * EPP])
            saet = eb.tile([128, S * 24], F32, tag="saet")
            nc.sync.dma_start(saet[:], saet_d[:, p0 * 24:(p0 + S) * 24])
            saetv = saet[:, :].rearrange("p (s l h) -> p s l h", l=3, h=H)

            hT_prev = None

            for li in (1, 2, 3):
                wch = wchunks[li]
                self_loops = li > 1
                first = li == 1
                sp = ps.tile([128, 384], F32, tag="sp")
                hAso = wk.tile([128, S * 272], vdt, tag="hAso")
                hAv = hAso[:, :].rearrange("p (pr x) -> p pr x", pr=S)
                if first and pend:
                    pool_head(pend)

                # ---- phase M: MM1 per pair + PSUM drain ----
                for j in range(S):
                    h2 = ps.tile([128, 272], F32, tag="h2", bufs=4)
                    if first:
                        stats = [xs[:, j * 128:(j + 1) * 128]]
                    else:
                        d, jj = j // 2, j % 2
                        hT = hT_prev[d]
                        stats = [hT[:, jj * 256:jj * 256 + 128],
                                 hT[:, jj * 256 + 128:jj * 256 + 256]]
                    for kc, (sta, wc) in enumerate(zip(stats, wch)):
                        nc.tensor.matmul(h2[:], sta, wc, start=(kc == 0),
                                         stop=(kc == len(wch) - 1))
                    copy_engine(HASO_ENG[j], hAso[:, j * 272:(j + 1) * 272],
                                h2[:])

                # ---- per half: fused gathers, dst scores, logit chain ----
                # (half-granular so exp(h0) sits early in the ACT queue and
                # p_e h0 is ready before the den matmuls reach the PE)
                msgraw = wk.tile([EPP, S * 264], vdt, tag="msgraw")
                mrv = msgraw[:, :].rearrange("p (pr x) -> p pr x", pr=S)
                p_e = wk.tile([EPP, 128], vdt, tag="p_e")
                for hf in (0, 1):
                    for j in range(hf * 8, hf * 8 + 8):
                        g2e = ps.tile([EPP, 264], F32, tag="h2", bufs=4)
                        nc.tensor.matmul(g2e[:], sblk[:, j * EPP:(j + 1) * EPP],
                                         hAv[:, j, 0:264], start=True, stop=True)
                        copy_engine(MRAW_ENG[j],
                                    msgraw[:, j * 264:(j + 1) * 264],
                                    g2e[:], fb_idx=j)
                    for j in range(hf * 8, hf * 8 + 8):
                        nc.tensor.matmul(sp[0:112, AD_ + j * 8:AD_ + j * 8 + 8],
                                         dblk[:, j * EPP:(j + 1) * EPP],
                                         hAv[:, j, 264:272],
                                         start=True, stop=False)
                        nc.tensor.matmul(sp[0:112, AD_ + j * 8:AD_ + j * 8 + 8],
                                         earow[0:1, j * EPP:(j + 1) * EPP],
                                         c_wev[0:1, (li - 1) * 8:li * 8],
                                         start=False, stop=True)
                    cs = slice(hf * 64, hf * 64 + 64)
                    lg = wk.tile([EPP, 64], F32, tag=f"lg{hf}")
                    nc.vector.scalar_tensor_tensor(
                        lg[:].rearrange("p (s h) -> p s h", h=H),
                        sp[0:112, AD_ + hf * 64:AD_ + hf * 64 + 64]
                        .rearrange("p (s h) -> p s h", h=H), 1.0,
                        mrv[:, hf * 8:(hf + 1) * 8, 256:264],
                        ALU.mult, ALU.add)
                    lg2 = wk.tile([EPP, 64], F32, tag=f"lg2{hf}")
                    nc.vector.scalar_tensor_tensor(
                        lg2[:], lg[:], 0.2, lg[:], ALU.mult, ALU.max)
                    nc.scalar.activation(p_e[:, cs], lg2[:], ACTF.Exp)

                # self-loop logit chain (full width, 16 pairs = 128 cols)
                p_self = None
                if self_loops:
                    s1 = wk.tile([128, 128], F32, tag="s1")
                    nc.gpsimd.tensor_tensor(
                        s1[:].rearrange("p (s h) -> p s h", h=H),
                        hAv[:, :, 256:264], hAv[:, :, 264:272], ALU.add)
                    s2 = wk.tile([128, 128], F32, tag="s2")
                    nc.gpsimd.tensor_tensor(
                        s2[:].rearrange("p (s h) -> p s h", h=H),
                        s1[:].rearrange("p (s h) -> p s h", h=H),
                        saetv[:, :, li - 1, :], ALU.add)
                    s3 = wk.tile([128, 128], F32, tag="s3")
                    nc.vector.scalar_tensor_tensor(
                        s3[:], s2[:], 0.2, s2[:], ALU.mult, ALU.max)
                    p_self = wk.tile([128, 128], F32, tag="p_self")
                    nc.scalar.activation(p_self[:], s3[:], ACTF.Exp)

                if li == 2 and pend:
                    pool_agent(pend)
                if li == 3 and pend:
                    pool_fc2(pend)

                # ---- denominators ----
                for j in range(S):
                    nc.tensor.matmul(sp[:, DEN_ + j * 8:DEN_ + j * 8 + 8],
                                     dtt[:, j * 128:(j + 1) * 128],
                                     p_e[:, j * 8:j * 8 + 8],
                                     start=True, stop=True)
                rden_v = wk.tile([128, 128], vdt, tag="rden_v")
                for hf in (0, 1):
                    cs = slice(hf * 64, hf * 64 + 64)
                    dtot = wk.tile([128, 64], F32, tag=f"dtot{hf}")
                    if self_loops:
                        nc.vector.tensor_tensor(
                            dtot[:], sp[:, DEN_ + hf * 64:DEN_ + hf * 64 + 64],
                            p_self[:, cs], ALU.add)
                    else:
                        nc.vector.tensor_scalar(
                            dtot[:], sp[:, DEN_ + hf * 64:DEN_ + hf * 64 + 64],
                            1e-16, None, ALU.add)
                    with nc.allow_low_precision(reason="attn denom in bf16"):
                        nc.vector.reciprocal(rden_v[:, cs], dtot[:])

                if li == 2 and pend:
                    pool_fc1g(pend)

                # ---- rd gather of 1/den at dst, pn/psn per duet ----
                pn = wk.tile([EPP, 128], vdt, tag="pn")
                psn = None
                if self_loops:
                    psn = wk.tile([128, 128], vdt, tag="psn")
                for j in range(S):
                    nc.tensor.matmul(sp[0:112, RD_ + j * 8:RD_ + j * 8 + 8],
                                     dblk[:, j * EPP:(j + 1) * EPP],
                                     rden_v[:, j * 8:j * 8 + 8],
                                     start=True, stop=True)
                    if j % 2 == 1:
                        ds = slice((j - 1) * 8, (j + 1) * 8)
                        nc.vector.tensor_tensor(
                            pn[:, ds], sp[0:112, RD_ + (j - 1) * 8:
                                          RD_ + (j + 1) * 8],
                            p_e[:, ds], ALU.mult)
                        if self_loops:
                            nc.gpsimd.tensor_tensor(psn[:, ds],
                                                    p_self[:, ds],
                                                    rden_v[:, ds], ALU.mult)

                # ---- value phase: prefetch messages, then scatter ----
                msgs = []
                for d in range(S // 2):
                    msg2 = wk.tile([EPP, 512], vdt, tag="msg2", bufs=8)
                    m_eng = MSG2_ENG[d] if self_loops else MSG2_L1[d]
                    tt_engine(
                        m_eng, msg2[:].rearrange(
                            "p (a h c) -> p a h c", a=2, h=H),
                        mrv[:, 2 * d:2 * d + 2, 0:256]
                        .rearrange("p a (h c) -> p a h c", h=H),
                        pn[:, d * 16:(d + 1) * 16]
                        .rearrange("p (a h) -> p a h", a=2)[:, :, :, None]
                        .broadcast_to([EPP, 2, H, HID]), ALU.mult)
                    msgs2 = None
                    if self_loops:
                        msgs2 = wk.tile([128, 512], vdt, tag="msgs2", bufs=8)
                        tt_engine(
                            MSGS_ENG[d], msgs2[:].rearrange(
                                "p (a h c) -> p a h c", a=2, h=H),
                            hAv[:, 2 * d:2 * d + 2, 0:256]
                            .rearrange("p a (h c) -> p a h c", h=H),
                            psn[:, d * 16:(d + 1) * 16]
                            .rearrange("p (a h) -> p a h", a=2)[:, :, :, None]
                            .broadcast_to([128, 2, H, HID]), ALU.mult)
                    if first and pend:
                        pool_gvv(pend, d)
                    msgs.append((msg2, msgs2))
                if first and pend:
                    pool_gev_cast(pend)
                hT_new = [None] * (S // 2)
                for d in range(S // 2):
                    msg2, msgs2 = msgs[d]
                    o2 = ps.tile([128, 512], F32, tag="o2")
                    for jj in range(2):
                        j = 2 * d + jj
                        for c in range(2):
                            cs = slice(jj * 256 + c * 128, jj * 256 + c * 128 + 128)
                            nc.tensor.matmul(
                                o2[:, cs],
                                msg2[:, jj * 256 + c * 128:jj * 256 + (c + 1) * 128],
                                dtt[:, j * 128:(j + 1) * 128],
                                start=True, stop=not self_loops)
                            if self_loops:
                                nc.tensor.matmul(
                                    o2[:, cs],
                                    msgs2[:, jj * 256 + c * 128:jj * 256 + (c + 1) * 128],
                                    c_id[:, :], start=False, stop=True)
                    hT2 = wk.tile([128, 512], vdt, tag=f"hT{d}")
                    r_eng = RELU_ENG[d] if self_loops else RELU_L1[d]
                    relu_engine(r_eng, hT2[:], o2[:], fb_idx=d)
                    hT_new[d] = hT2
                hT_prev = hT_new

            pend = {"hT": hT_prev, "st": st}

        # flush final supertile's pooling
        pool_head(pend)
        for d in range(S // 2):
            pool_gvv(pend, d)
        pool_gev_cast(pend)
        pool_agent(pend)
        pool_fc1g(pend)
        pool_fc2(pend)

    nc.compile()
    return nc


# ---------------- host-side packing ----------------

def _np_vdt(vdt):
    import ml_dtypes
    return {mybir.dt.bfloat16: ml_dtypes.bfloat16,
            mybir.dt.float32: np.float32}[vdt]


def host_prep(inputs, npairs=GPC // 2, vdt=mybir.dt.bfloat16):
    nv = _np_vdt(vdt)
    x = np.asarray(inputs["x"], np.float32)
    ei = np.asarray(inputs["edge_index"])
    eattr = np.asarray(inputs["edge_attr"], np.float32)
    for l in (1, 2, 3):
        assert not np.any(np.asarray(inputs[f"b{l}"])), "GAT bias must be 0"

    def pack_w(l):
        W = np.asarray(inputs[f"W{l}"], np.float32)
        a_s = np.asarray(inputs[f"as{l}"], np.float32)
        a_d = np.asarray(inputs[f"ad{l}"], np.float32)
        Ps = np.einsum("fkc,kc->fk", W.reshape(W.shape[0], H, HID), a_s)
        Pd = np.einsum("fkc,kc->fk", W.reshape(W.shape[0], H, HID), a_d)
        return np.concatenate([W, Ps, Pd], axis=1).astype(nv)

    def w_e(l):
        We = np.asarray(inputs[f"We{l}"], np.float32).reshape(H, HID)
        a_e = np.asarray(inputs[f"ae{l}"], np.float32)
        return (We * a_e).sum(-1)

    waug = {l: pack_w(l) for l in (1, 2, 3)}
    for l in (2, 3):
        waug[l] = np.concatenate([waug[l][:128], waug[l][128:]], axis=1)
    wev3 = np.stack([w_e(l) for l in (1, 2, 3)])   # [3, 8]
    fc1_w = np.asarray(inputs["fc1_w"], np.float32)
    fc1a = np.concatenate([fc1_w[:128], fc1_w[128:HC]], axis=1).astype(nv)
    fc1g = np.concatenate([fc1_w[HC:HC + 128] / P,
                           fc1_w[HC + 128:] / P], axis=1).astype(nv)
    fc1b = np.asarray(inputs["fc1_b"], np.float32).reshape(128, 1)
    fc2w = np.asarray(inputs["fc2_w"], np.float32).astype(nv)
    fc2b = np.asarray(inputs["fc2_b"], np.float32).reshape(OUT, 1)
    identm = np.eye(128, dtype=np.float32).astype(nv)
    wevr = wev3.reshape(1, 24).astype(nv)

    maps = []
    npc = GPC * P
    epc = GPC * OBS
    for m in range(NCORES):
        nsl = slice(m * npc, (m + 1) * npc)
        esl = slice(m * epc, (m + 1) * epc)
        xtm = np.ascontiguousarray(x[nsl].T).astype(nv)
        src = np.asarray(ei[0][esl], np.int64) - m * npc
        dst = np.asarray(ei[1][esl], np.int64) - m * npc
        pairs = np.arange(GPC // 2).repeat(EPP)
        src_l = (src.reshape(-1) - pairs * 128).astype(np.int64).reshape(npairs, EPP)
        dst_l = (dst.reshape(-1) - pairs * 128).astype(np.int64).reshape(npairs, EPP)
        ea_pair = eattr[esl].reshape(npairs, EPP)

        # one-hot masks
        P_, E_ = np.indices((npairs, EPP))
        sblk = np.zeros((128, npairs, EPP), np.float32)
        sblk[src_l, P_, E_] = 1.0
        dblk = np.zeros((128, npairs, EPP), np.float32)
        dblk[dst_l, P_, E_] = 1.0
        dttm = np.zeros((EPP, npairs, 128), np.float32)
        dttm[E_.T, P_.T, dst_l.T] = 1.0

        # self-loop edge_attr mean per node
        cnt = np.zeros((npairs, 128), np.float32)
        np.add.at(cnt, (P_, dst_l), 1.0)
        easum = np.zeros((npairs, 128), np.float32)
        np.add.at(easum, (P_, dst_l), ea_pair)
        ea_loop = easum / np.maximum(cnt, 1.0)     # [npairs, 128]

        saetm = (ea_loop.T[:, :, None, None] * wev3[None, None, :, :])

        maps.append({
            "xt": xtm[:, :npairs * 128],
            "sblk": np.ascontiguousarray(
                sblk.reshape(128, npairs * EPP)).astype(nv),
            "dblk": np.ascontiguousarray(
                dblk.reshape(128, npairs * EPP)).astype(nv),
            "dtt": np.ascontiguousarray(
                dttm.reshape(EPP, npairs * 128)).astype(nv),
            "earow": ea_pair.reshape(1, npairs * EPP).astype(nv),
            "saet": np.ascontiguousarray(
                saetm.reshape(128, npairs * 24)).astype(np.float32),
            "wev": wevr,
            "waug1": waug[1], "waug2": waug[2], "waug3": waug[3],
            "fc1a": fc1a, "fc1g": fc1g, "fc1b": fc1b,
            "fc2w": fc2w, "fc2b": fc2b, "ident": identm,
        })
    return maps


def unpack_out(res_list, npairs=GPC // 2):
    outs = []
    for m in range(NCORES):
        o = res_list[m]["out"]
        o = o.reshape(OUT, npairs // S, 2 * S, A).transpose(1, 2, 3, 0)
        outs.append(o.reshape(npairs * 2, A, OUT))
    return np.concatenate(outs, axis=0).astype(np.float32)


# ---------------- entry point ----------------

LAST_EXEC_NS = None
LAST_TRACE = None
_NC_CACHE = {}


def _install_trace_hook():
    """Best-effort: register the axon NTFF profile hook so trace=True works.

    The agent image's antenv lacks axon_hooks; fabricate it and wire the
    ctypes hook from trn_agent_boot. Silently a no-op anywhere else.
    """
    try:
        import sys
        import types
        if 'antenv.axon_hooks' not in sys.modules:
            import antenv
            mod = types.ModuleType('antenv.axon_hooks')
            _h = [None]
            mod.set_axon_ntff_profile_hook = lambda h: _h.__setitem__(0, h)
            mod.get_axon_ntff_profile_hook = lambda: _h[0]
            sys.modules['antenv.axon_hooks'] = mod
            antenv.axon_hooks = mod
        import antenv.axon_hooks as ah
        if ah.get_axon_ntff_profile_hook() is None:
            if '/root/.axon_site' not in sys.path:
                sys.path.insert(0, '/root/.axon_site')
            from trn_agent_boot.trn_boot import _ntff_profile_via_ctypes
            hook = _ntff_profile_via_ctypes('/opt/axon/libaxon_pjrt.so')
            if hook is not None:
                ah.set_axon_ntff_profile_hook(hook)
    except Exception:
        pass


def kernel(**inputs) -> np.ndarray:
    """Full-input GAT forward on 8 NeuronCores; returns [4096, 8, 2] f32."""
    global LAST_EXEC_NS, LAST_TRACE
    import os
    vdt = mybir.dt.bfloat16
    npairs = GPC // 2
    key = (npairs, vdt)
    if key not in _NC_CACHE:
        _NC_CACHE[key] = build(npairs, vdt=vdt, num_devices=NCORES)
    nc = _NC_CACHE[key]
    maps = host_prep(inputs, npairs=npairs, vdt=vdt)
    trace = os.environ.get("BASS_GAT_TRACE") == "1"
    if trace:
        _install_trace_hook()
    res = None
    for attempt in range(3):
        try:
            res = run_bass_kernel_spmd(
                nc, maps, core_ids=list(range(NCORES)),
                trace=trace and attempt == 0,
                trace_cores=[0] if trace and attempt == 0 else None)
            break
        except Exception as e:
            import traceback
            print(f"kernel attempt {attempt} failed: {type(e).__name__}: "
                  f"{str(e)[:500]}")
            if os.environ.get("BASS_GAT_VERBOSE") == "1":
                traceback.print_exc()
            if attempt == 2:
                raise
            import time
            time.sleep(10)
    LAST_EXEC_NS = res.exec_time_ns
    LAST_TRACE = res.instructions_and_trace
    return unpack_out([r for r in res.results], npairs=npairs)


# revision 24
# speedup vs baseline: 1.1629x; 1.1629x over previous
"""GAT model Bass/Tile kernel for TRN2 (self-contained, octet-batched).

Per core: 512 graphs as 256 pairs (128 nodes / 112 edges). Pairs are
processed in octets (8 pairs): per-edge/per-node attention scalars are
batched into [*, 64] ops across the octet; fat value ops run at duet
(2-pair) granularity; engines are balanced DVE/ACT/GPSIMD/PE.
"""
import numpy as np
from contextlib import ExitStack

import concourse.bass as bass
import concourse.tile as tile
from concourse import bacc, mybir
from concourse.bass_utils import run_bass_kernel_spmd

F32 = mybir.dt.float32
I32 = mybir.dt.int32

B, A, OBS = 4096, 8, 56
P = 64
H, HID, HC = 8, 32, 256
IN, OUT = 16, 2
NCORES = 8
GPC = B // NCORES
EPP = 2 * OBS
ALU = mybir.AluOpType
ACTF = mybir.ActivationFunctionType

# small_ps column regions (f32); Z/ZG/oc reuse the same tile post-L3
ASD_, DEN_, RD_, CNT_, Z_, ZG_ = 0, 64, 128, 192, 0, 128


def build(npairs: int, vdt=mybir.dt.bfloat16, num_devices: int = NCORES):
    assert npairs % 8 == 0
    nc = bacc.Bacc("TRN2", target_bir_lowering=False, debug=False,
                   num_devices=num_devices)
    NP = npairs

    def din(name, shape, dt):
        return nc.dram_tensor(name, shape, dt, kind="ExternalInput").ap()

    xt = din("xt", [IN, NP * 128], vdt)
    esrcb = din("esrcb", [NP, EPP], vdt)
    edstb = din("edstb", [NP, EPP], vdt)
    edst = din("edst", [EPP, NP], F32)
    eattr = din("eattr", [EPP, NP], F32)
    eap = din("eap", [EPP, 2 * NP], vdt)
    waug1 = din("waug1", [IN, 272], vdt)
    waug2 = din("waug2", [128, 544], vdt)
    waug3 = din("waug3", [128, 544], vdt)
    webe = din("webe", [EPP, 3 * 64], F32)    # w_e tiled 8x per layer
    webn = din("webn", [128, 3 * 64], F32)
    fc1a = din("fc1a", [128, HC], vdt)
    fc1g = din("fc1g", [128, HC], vdt)
    fc1b = din("fc1b", [128, 1], F32)
    fc2w = din("fc2w", [128, OUT], vdt)
    fc2b = din("fc2b", [OUT, 1], F32)
    ident = din("ident", [128, 128], vdt)
    iota = din("iota", [EPP, 128], vdt)
    iotac = din("iotac", [128, 1], F32)

    out_d = nc.dram_tensor("out", [OUT, NP * 16], F32, kind="ExternalOutput").ap()

    with tile.TileContext(nc) as tc, ExitStack() as ctx:
        cpool = ctx.enter_context(tc.tile_pool(name="const", bufs=1))
        wk = ctx.enter_context(tc.tile_pool(name="work", bufs=4))
        eb = ctx.enter_context(tc.tile_pool(name="edges", bufs=24))
        ps = ctx.enter_context(tc.tile_pool(name="psum", bufs=1, space="PSUM"))

        def cload(ap, tag):
            t = cpool.tile(list(ap.shape), ap.dtype, tag=tag)
            nc.sync.dma_start(t[:], ap[:, :])
            return t

        c_w1, c_w2, c_w3 = cload(waug1, "w1"), cload(waug2, "w2"), cload(waug3, "w3")
        c_webe, c_webn = cload(webe, "webe"), cload(webn, "webn")
        c_fc1a, c_fc1g = cload(fc1a, "fc1a"), cload(fc1g, "fc1g")
        c_fc1b, c_fc2w, c_fc2b = cload(fc1b, "fc1b"), cload(fc2w, "fc2w"), cload(fc2b, "fc2b")
        c_id, c_iota = cload(ident, "ident"), cload(iota, "iota")
        c_iotac = cload(iotac, "iotac")
        c_edst = cload(edst, "edst")
        c_ea, c_eap = cload(eattr, "eattr"), cload(eap, "eap")

        out_acc = cpool.tile([OUT, NP * 16], F32, tag="out_acc")

        wchunks = {1: [c_w1[:, :]],
                   2: [c_w2[:, 0:272], c_w2[:, 272:544]],
                   3: [c_w3[:, 0:272], c_w3[:, 272:544]]}

        for oct_i in range(NP // 8):
            p0 = oct_i * 8

            # ---- phase A: edge structure + x loads ----
            srcb = eb.tile([128, 8 * EPP], vdt, tag="srcb", bufs=2)
            nc.sync.dma_start(srcb[:], esrcb[p0:p0 + 8, :]
                              .rearrange("a b -> (a b)")[None, :]
                              .broadcast_to([128, 8 * EPP]))
            dstb = eb.tile([128, 8 * EPP], vdt, tag="dstb", bufs=2)
            nc.sync.dma_start(dstb[:], edstb[p0:p0 + 8, :]
                              .rearrange("a b -> (a b)")[None, :]
                              .broadcast_to([128, 8 * EPP]))
            sblk_o = eb.tile([128, 8 * EPP], vdt, tag="sblk_o", bufs=2)
            nc.vector.tensor_scalar(sblk_o[:], srcb[:], c_iotac[:, 0:1],
                                    None, ALU.is_equal)
            dblk_o = eb.tile([128, 8 * EPP], vdt, tag="dblk_o", bufs=2)
            nc.vector.tensor_scalar(dblk_o[:], dstb[:], c_iotac[:, 0:1],
                                    None, ALU.is_equal)
            sblk_l = [sblk_o[:, j * EPP:(j + 1) * EPP] for j in range(8)]
            dblk_l = [dblk_o[:, j * EPP:(j + 1) * EPP] for j in range(8)]
            dtt_l, x0_l = [], []
            for j in range(8):
                pp = p0 + j
                dtt = eb.tile([EPP, 128], vdt, tag="dtt")
                nc.vector.tensor_scalar(dtt[:], c_iota[:], c_edst[:, pp:pp + 1],
                                        None, ALU.is_equal)
                x0 = eb.tile([IN, 128], vdt, tag="x0")
                nc.sync.dma_start(x0[:], xt[:, pp * 128:(pp + 1) * 128])
                dtt_l.append(dtt); x0_l.append(x0)

            prevT = [[x0_l[j][:, :]] for j in range(8)]
            ea_loop = None
            hT_l3 = None

            for li in (1, 2, 3):
                wch = wchunks[li]
                self_loops = li > 1
                sp = ps.tile([128, 208], F32, tag="small", bufs=2)
                hAso = wk.tile([128, 8 * 272], vdt, tag="hAso", bufs=4)
                hAv = hAso[:, :].rearrange("p (pr x) -> p pr x", pr=8)

                # ---- MM1 per pair + hAs copy + small gathers ----
                for j in range(8):
                    h2 = ps.tile([128, 272], F32, tag="h2", bufs=3)
                    for kc, (sta, wc) in enumerate(zip(prevT[j], wch)):
                        nc.tensor.matmul(h2[:], sta, wc, start=(kc == 0),
                                         stop=(kc == len(wch) - 1))
                    if j % 4 == 0:
                        nc.vector.tensor_copy(
                            hAso[:, j * 272:(j + 1) * 272], h2[:])
                    else:
                        nc.scalar.copy(
                            hAso[:, j * 272:(j + 1) * 272], h2[:])
                    nc.tensor.matmul(sp[0:112, ASD_ + j * 8:ASD_ + j * 8 + 8],
                                     sblk_l[j], hAv[:, j, 256:264],
                                     start=True, stop=False)
                    nc.tensor.matmul(sp[0:112, ASD_ + j * 8:ASD_ + j * 8 + 8],
                                     dblk_l[j], hAv[:, j, 264:272],
                                     start=False, stop=True)
                    if li == 1:
                        nc.tensor.matmul(
                            sp[:, CNT_ + j * 2:CNT_ + j * 2 + 2], dtt_l[j][:],
                            c_eap[:, 2 * (p0 + j):2 * (p0 + j) + 2],
                            start=True, stop=True)

                if li == 1:
                    cntv = sp[:, CNT_:CNT_ + 16].rearrange(
                        "p (pr two) -> p pr two", two=2)
                    cntm = wk.tile([128, 8], F32, tag="cntm")
                    nc.vector.tensor_scalar(cntm[:], cntv[:, :, 1:2], 1.0,
                                            None, ALU.max)
                    rc = wk.tile([128, 8], F32, tag="rc")
                    nc.vector.reciprocal(rc[:], cntm[:])
                    ea_loop = wk.tile([128, 8], F32, tag="ea_loop")
                    nc.vector.tensor_tensor(ea_loop[:], cntv[:, :, 0:1]
                                            .rearrange("p a b -> p (a b)"),
                                            rc[:], ALU.mult)

                # ---- batched edge logits ----
                ae = wk.tile([EPP, 64], F32, tag="ae")
                nc.gpsimd.tensor_tensor(
                    ae[:].rearrange("p (a h) -> p a h", a=8),
                    c_webe[:, (li - 1) * 64:li * 64]
                    .rearrange("p (a h) -> p a h", a=8),
                    c_ea[:, p0:p0 + 8][:, :, None].broadcast_to([EPP, 8, H]),
                    ALU.mult)
                lg = wk.tile([EPP, 64], F32, tag="lg")
                nc.vector.scalar_tensor_tensor(
                    lg[:], sp[0:112, ASD_:ASD_ + 64], 1.0, ae[:],
                    ALU.mult, ALU.add)
                lg2 = wk.tile([EPP, 64], F32, tag="lg2")
                nc.vector.scalar_tensor_tensor(
                    lg2[:], lg[:], 0.2, lg[:], ALU.mult, ALU.max)
                p_e = wk.tile([EPP, 64], vdt, tag="p_e")
                nc.scalar.activation(p_e[:], lg2[:], ACTF.Exp)

                p_self = None
                if self_loops:
                    sae = wk.tile([128, 64], F32, tag="sae")
                    nc.gpsimd.tensor_tensor(
                        sae[:].rearrange("p (a h) -> p a h", a=8),
                        c_webn[:, (li - 1) * 64:li * 64]
                        .rearrange("p (a h) -> p a h", a=8),
                        ea_loop[:][:, :, None].broadcast_to([128, 8, H]),
                        ALU.mult)
                    s1 = wk.tile([128, 64], F32, tag="s1")
                    nc.gpsimd.tensor_tensor(
                        s1[:].rearrange("p (a h) -> p a h", a=8),
                        hAv[:, :, 256:264],
                        hAv[:, :, 264:272], ALU.add)
                    s2 = wk.tile([128, 64], F32, tag="s2")
                    nc.gpsimd.tensor_tensor(
                        s2[:], s1[:], sae[:], ALU.add)
                    s3 = wk.tile([128, 64], F32, tag="s3")
                    nc.vector.scalar_tensor_tensor(
                        s3[:], s2[:], 0.2, s2[:], ALU.mult, ALU.max)
                    p_self = wk.tile([128, 64], F32, tag="p_self")
                    nc.scalar.activation(p_self[:], s3[:], ACTF.Exp)

                # ---- denominators ----
                for j in range(8):
                    nc.tensor.matmul(sp[:, DEN_ + j * 8:DEN_ + j * 8 + 8],
                                     dtt_l[j][:], p_e[:, j * 8:j * 8 + 8],
                                     start=True, stop=True)
                dtot = wk.tile([128, 64], F32, tag="dtot")
                if self_loops:
                    nc.vector.tensor_tensor(dtot[:], sp[:, DEN_:DEN_ + 64],
                                            p_self[:], ALU.add)
                else:
                    nc.vector.tensor_scalar(dtot[:], sp[:, DEN_:DEN_ + 64],
                                            1e-16, None, ALU.add)
                rden = wk.tile([128, 64], F32, tag="rden")
                nc.vector.reciprocal(rden[:], dtot[:])
                rden_v = wk.tile([128, 64], vdt, tag="rden_v")
                nc.scalar.copy(rden_v[:], rden[:])
                for j in range(8):
                    nc.tensor.matmul(sp[0:112, RD_ + j * 8:RD_ + j * 8 + 8],
                                     dblk_l[j], rden_v[:, j * 8:j * 8 + 8],
                                     start=True, stop=True)
                pn = wk.tile([EPP, 64], vdt, tag="pn")
                nc.vector.tensor_tensor(pn[:], sp[0:112, RD_:RD_ + 64], p_e[:],
                                        ALU.mult)
                if self_loops:
                    psn = wk.tile([128, 64], vdt, tag="psn")
                    nc.gpsimd.tensor_tensor(psn[:], p_self[:], rden[:], ALU.mult)

                # ---- phase B per duet: gather h, messages, scatter, relu ----
                hT_new = []
                for d in range(4):
                    g2 = ps.tile([EPP, 512], F32, tag="g2", bufs=2)
                    for jj in range(2):
                        j = 2 * d + jj
                        nc.tensor.matmul(g2[:, jj * 256:jj * 256 + 256],
                                         sblk_l[j], hAv[:, j, 0:256],
                                         start=True, stop=True)
                    msg2 = wk.tile([EPP, 512], vdt, tag="msg2")
                    nc.vector.tensor_tensor(
                        msg2[:].rearrange("p (a h c) -> p a h c", a=2, h=H),
                        g2[:, :].rearrange("p (a h c) -> p a h c", a=2, h=H),
                        pn[:, d * 16:(d + 1) * 16]
                        .rearrange("p (a h) -> p a h", a=2)[:, :, :, None]
                        .broadcast_to([EPP, 2, H, HID]), ALU.mult)
                    if self_loops:
                        msgs2 = wk.tile([128, 512], vdt, tag="msgs2")
                        for jj in range(2):
                            j = 2 * d + jj
                            nc.gpsimd.tensor_tensor(
                                msgs2[:, jj * 256:(jj + 1) * 256]
                                .rearrange("p (h c) -> p h c", h=H),
                                hAv[:, j, 0:256]
                                .rearrange("p (h c) -> p h c", h=H),
                                psn[:, j * 8:(j + 1) * 8][:, :, None]
                                .broadcast_to([128, H, HID]), ALU.mult)
                    o2 = ps.tile([128, 512], F32, tag="out2", bufs=1)
                    for jj in range(2):
                        j = 2 * d + jj
                        for c in range(2):
                            cs = slice(jj * 256 + c * 128, jj * 256 + c * 128 + 128)
                            nc.tensor.matmul(
                                o2[:, cs], msg2[:, jj * 256 + c * 128:
                                                jj * 256 + (c + 1) * 128],
                                dtt_l[j][:], start=True, stop=not self_loops)
                            if self_loops:
                                nc.tensor.matmul(
                                    o2[:, cs], msgs2[:, jj * 256 + c * 128:
                                                     jj * 256 + (c + 1) * 128],
                                    c_id[:, :], start=False, stop=True)
                    hT2 = wk.tile([128, 512], vdt, tag=f"hT{li}_{d}")
                    nc.scalar.activation(hT2[:], o2[:], ACTF.Relu)
                    hT_new.append(hT2)
                    for jj in range(2):
                        j = 2 * d + jj
                        prevT[j] = [hT2[:, jj * 256:jj * 256 + 128],
                                    hT2[:, jj * 256 + 128:jj * 256 + 256]]
                hT_l3 = hT_new

            # ---- pooling + MLP (octet-batched) ----
            zmlp = ps.tile([128, 144], F32, tag="g2", bufs=2)
            gev_o = wk.tile([128, 32], F32, tag="gev_o")   # (c, pair, g)
            gvv = gev_o[:, :].rearrange("p (c pr g) -> p pr c g", c=2, g=2)
            for d in range(4):
                nc.vector.tensor_reduce(
                    gvv[:, 2 * d:2 * d + 2, :, :],
                    hT_l3[d][:, :].rearrange(
                        "p (a b g n) -> p a b g n", a=2, b=2, g=2),
                    mybir.AxisListType.X, ALU.add)
            gev_v = wk.tile([128, 32], vdt, tag="gev_v")
            nc.scalar.copy(gev_v[:], gev_o[:])
            for d in range(4):
                for c in range(2):
                    agent = hT_l3[d][:, :].rearrange(
                        "p (a b g n) -> p a b g n", a=2, b=2, g=2)[:, :, c, :, 0:8]
                    nc.tensor.matmul(zmlp[:, Z_ + d * 32:Z_ + (d + 1) * 32],
                                     c_fc1a[:, bass.ts(c, 128)], agent,
                                     start=(c == 0), stop=(c == 1))
            for c in range(2):
                nc.tensor.matmul(zmlp[:, ZG_:ZG_ + 16],
                                 c_fc1g[:, bass.ts(c, 128)],
                                 gev_v[:, c * 16:(c + 1) * 16],
                                 start=(c == 0), stop=(c == 1))
            zgb = wk.tile([128, 16], F32, tag="zgb")
            nc.vector.scalar_tensor_tensor(
                zgb[:], zmlp[:, ZG_:ZG_ + 16], 1.0,
                c_fc1b[:, 0:1].broadcast_to([128, 16]), ALU.mult, ALU.add)
            zt = wk.tile([128, 128], F32, tag="zt")
            nc.vector.scalar_tensor_tensor(
                zt[:].rearrange("p (a b) -> p a b", a=16),
                zmlp[:, Z_:Z_ + 128].rearrange("p (a b) -> p a b", a=16), 1.0,
                zgb[:][:, :, None].broadcast_to([128, 16, 8]),
                ALU.mult, ALU.add)
            zbat = wk.tile([128, 128], vdt, tag="zbat")
            nc.scalar.activation(zbat[:], zt[:], ACTF.Relu)
            nc.tensor.matmul(zmlp[0:OUT, 0:128], c_fc2w[:, :], zbat[:],
                             start=True, stop=True)
            nc.vector.tensor_scalar(out_acc[:, oct_i * 128:(oct_i + 1) * 128],
                                    zmlp[0:OUT, 0:128], c_fc2b[:, 0:1], None,
                                    ALU.add)

        nc.sync.dma_start(out_d[:, :], out_acc[:])

    nc.compile()
    return nc


# ---------------- host-side packing ----------------

def _np_vdt(vdt):
    import ml_dtypes
    return {mybir.dt.bfloat16: ml_dtypes.bfloat16,
            mybir.dt.float32: np.float32}[vdt]


def host_prep(inputs, npairs=GPC // 2, vdt=mybir.dt.bfloat16):
    nv = _np_vdt(vdt)
    x = np.asarray(inputs["x"], np.float32)
    ei = np.asarray(inputs["edge_index"])
    eattr = np.asarray(inputs["edge_attr"], np.float32)
    for l in (1, 2, 3):
        assert not np.any(np.asarray(inputs[f"b{l}"])), "GAT bias must be 0"

    def pack_w(l):
        W = np.asarray(inputs[f"W{l}"], np.float32)
        a_s = np.asarray(inputs[f"as{l}"], np.float32)
        a_d = np.asarray(inputs[f"ad{l}"], np.float32)
        Ps = np.einsum("fkc,kc->fk", W.reshape(W.shape[0], H, HID), a_s)
        Pd = np.einsum("fkc,kc->fk", W.reshape(W.shape[0], H, HID), a_d)
        return np.concatenate([W, Ps, Pd], axis=1).astype(nv)

    def w_e(l):
        We = np.asarray(inputs[f"We{l}"], np.float32).reshape(H, HID)
        a_e = np.asarray(inputs[f"ae{l}"], np.float32)
        return (We * a_e).sum(-1)

    waug = {l: pack_w(l) for l in (1, 2, 3)}
    for l in (2, 3):
        waug[l] = np.concatenate([waug[l][:128], waug[l][128:]], axis=1)
    wev = np.concatenate([np.tile(w_e(l), 8) for l in (1, 2, 3)])   # [192]
    webe = np.broadcast_to(wev, (EPP, 192)).astype(np.float32).copy()
    webn = np.broadcast_to(wev, (128, 192)).astype(np.float32).copy()
    fc1_w = np.asarray(inputs["fc1_w"], np.float32)
    fc1a = np.concatenate([fc1_w[:128], fc1_w[128:HC]], axis=1).astype(nv)
    fc1g = np.concatenate([fc1_w[HC:HC + 128] / P,
                           fc1_w[HC + 128:] / P], axis=1).astype(nv)
    fc1b = np.asarray(inputs["fc1_b"], np.float32).reshape(128, 1)
    fc2w = np.asarray(inputs["fc2_w"], np.float32).astype(nv)
    fc2b = np.asarray(inputs["fc2_b"], np.float32).reshape(OUT, 1)
    identm = np.eye(128, dtype=np.float32).astype(nv)
    iota = np.broadcast_to(np.arange(128, dtype=np.float32),
                           (EPP, 128)).astype(nv).copy()
    iotac = np.arange(128, dtype=np.float32).reshape(128, 1)

    maps = []
    npc = GPC * P
    epc = GPC * OBS
    for m in range(NCORES):
        nsl = slice(m * npc, (m + 1) * npc)
        esl = slice(m * epc, (m + 1) * epc)
        xt = np.ascontiguousarray(x[nsl].T).astype(nv)
        src = np.asarray(ei[0][esl], np.int64) - m * npc
        dst = np.asarray(ei[1][esl], np.int64) - m * npc
        pairs = np.arange(GPC // 2).repeat(EPP)
        src_l = (src.reshape(-1) - pairs * 128).astype(np.float32)
        dst_l = (dst.reshape(-1) - pairs * 128).astype(np.float32)
        esrcb = np.ascontiguousarray(src_l.reshape(-1, EPP)).astype(nv)
        edstb = np.ascontiguousarray(dst_l.reshape(-1, EPP)).astype(nv)
        edst = np.ascontiguousarray(dst_l.reshape(-1, EPP).T)
        eat = np.ascontiguousarray(eattr[esl].reshape(-1, EPP).T).astype(np.float32)
        eap_arr = np.empty((EPP, 2 * npairs), np.float32)
        eap_arr[:, 0::2] = eat[:, :npairs]
        eap_arr[:, 1::2] = 1.0
        maps.append({
            "xt": xt[:, :npairs * 128],
            "esrcb": esrcb[:npairs], "edstb": edstb[:npairs],
            "edst": edst[:, :npairs],
            "eattr": eat[:, :npairs], "eap": eap_arr.astype(nv),
            "waug1": waug[1], "waug2": waug[2], "waug3": waug[3],
            "webe": webe, "webn": webn,
            "fc1a": fc1a, "fc1g": fc1g, "fc1b": fc1b,
            "fc2w": fc2w, "fc2b": fc2b,
            "ident": identm, "iota": iota, "iotac": iotac,
        })
    return maps


def unpack_out(res_list, npairs=GPC // 2):
    outs = []
    for m in range(NCORES):
        o = res_list[m]["out"]
        o = o.reshape(OUT, npairs, 2, A).transpose(1, 2, 3, 0)
        outs.append(o.reshape(npairs * 2, A, OUT))
    return np.concatenate(outs, axis=0).astype(np.float32)


# ---------------- entry point ----------------

LAST_EXEC_NS = None
LAST_TRACE = None
_NC_CACHE = {}


def _install_trace_hook():
    """Best-effort: register the axon NTFF profile hook so trace=True works.

    The agent image's antenv lacks axon_hooks; fabricate it and wire the
    ctypes hook from trn_agent_boot. Silently a no-op anywhere else.
    """
    try:
        import sys
        import types
        if 'antenv.axon_hooks' not in sys.modules:
            import antenv
            mod = types.ModuleType('antenv.axon_hooks')
            _h = [None]
            mod.set_axon_ntff_profile_hook = lambda h: _h.__setitem__(0, h)
            mod.get_axon_ntff_profile_hook = lambda: _h[0]
            sys.modules['antenv.axon_hooks'] = mod
            antenv.axon_hooks = mod
        import antenv.axon_hooks as ah
        if ah.get_axon_ntff_profile_hook() is None:
            if '/root/.axon_site' not in sys.path:
                sys.path.insert(0, '/root/.axon_site')
            from trn_agent_boot.trn_boot import _ntff_profile_via_ctypes
            hook = _ntff_profile_via_ctypes('/opt/axon/libaxon_pjrt.so')
            if hook is not None:
                ah.set_axon_ntff_profile_hook(hook)
    except Exception:
        pass


def kernel(**inputs) -> np.ndarray:
    """Full-input GAT forward on 8 NeuronCores; returns [4096, 8, 2] f32."""
    global LAST_EXEC_NS, LAST_TRACE
    import os
    vdt = mybir.dt.bfloat16
    npairs = GPC // 2
    key = (npairs, vdt)
    if key not in _NC_CACHE:
        _NC_CACHE[key] = build(npairs, vdt=vdt, num_devices=NCORES)
    nc = _NC_CACHE[key]
    maps = host_prep(inputs, npairs=npairs, vdt=vdt)
    trace = os.environ.get("BASS_GAT_TRACE") == "1"
    if trace:
        _install_trace_hook()
    res = None
    for attempt in range(3):
        try:
            res = run_bass_kernel_spmd(
                nc, maps, core_ids=list(range(NCORES)),
                trace=trace and attempt == 0,
                trace_cores=[0] if trace and attempt == 0 else None)
            break
        except Exception:
            if attempt == 2:
                raise
            import time
            time.sleep(10)
    LAST_EXEC_NS = res.exec_time_ns
    LAST_TRACE = res.instructions_and_trace
    return unpack_out([r for r in res.results], npairs=npairs)
